# revision 43
# baseline (speedup 1.0000x reference)
"""GroupedQueryAttention Trainium2 kernel (8 NeuronCores).

Sharding: core i handles (batch b = i//4, KV group g = i%4): its 4 query
heads + 1 KV group, full sequence. Each core computes a partial output
(attn_heads @ Wo rows for its heads); host sums the 4 partials per batch.

v2 layout strategy (per core), all matmul operands bf16 (fp32 PSUM):
  - x resident in SBUF as [128, e-chunk, t] bf16; weights pre-arranged on
    host into exact SBUF layouts so each loads with ONE contiguous DMA.
  - projections W-stationary: qT/kT [d, t] accumulated over 16 e-chunks.
  - RoPE: host permutes W rows to half-split layout; swap-half via a
    permutation matmul on PE; cos/sin tables applied on DVE (bf16 2x).
  - attention interleaved with projections (attention block bi runs right
    after projection block tb=bi): scoresT [s, tq] = kT_tile^T @ qT,
    exp on ACT (scores bounded by construction; no max subtraction),
    denominators via ones-matmul, PV with v[s,d]-stationary accumulation
    -> outT [d, tq]; diagonal blocks narrowed to the causal width.
  - out projection: attn tiles stationary, Wo chunks moving; results
    copied to bf16 SBUF (DVE/ACT alternating) and DMAd per 128-row stripe.
"""

import numpy as np
import ml_dtypes
from contextlib import ExitStack

import concourse.bass as bass
import concourse.bacc as bacc
import concourse.tile as tile
import concourse.mybir as mybir
from concourse.bass_utils import run_bass_kernel_spmd

# problem shape (hardcoded per contract)
B, T, E = 2, 2048, 2048
NH, NG, HD = 16, 4, 128
HPG = NH // NG          # 4 heads per group = per core
NE = E // 128           # 16 contraction chunks
TB = 512                # tq / t block
NTB = T // TB           # 4
NST = T // 128          # 16 t-tiles
F32 = mybir.dt.float32
BF16 = mybir.dt.bfloat16
EXP = mybir.ActivationFunctionType.Exp
COPY = mybir.ActivationFunctionType.Copy

N_CORES = 8
BF = ml_dtypes.bfloat16


def build_body(tc, out_ap, ins):
    """ins: dict name -> dram AP. out_ap: [T, E] bf16 dram AP."""
    nc = tc.nc
    ctx = ExitStack()
    with ctx:
        ctx.enter_context(nc.allow_low_precision(
            reason="bf16 matmul inputs / bf16 intermediate rounding is intended"))
        # ---- constant / persistent SBUF ----
        const = ctx.enter_context(tc.tile_pool(name="const", bufs=1))
        cs2 = const.tile([128, T], BF16, tag="cs2", name="cs2")
        snpm = const.tile([128, T], BF16, tag="snpm", name="snpm")
        tri = const.tile([128, 128], BF16, tag="tri", name="tri")
        swp = const.tile([128, 128], BF16, tag="swp", name="swp")
        iden = const.tile([128, 128], BF16, tag="iden", name="iden")
        ones = const.tile([128, 128], BF16, tag="ones", name="ones")

        persist = ctx.enter_context(tc.tile_pool(name="persist", bufs=1))
        xblk = [persist.tile([128, NE * TB], BF16, tag=f"xb{t}", name=f"xb{t}")
                for t in range(NTB)]
        qrot = [persist.tile([128, T], BF16, tag=f"qrot{h}", name=f"qrot{h}")
                for h in range(HPG)]
        aout = [persist.tile([128, T], BF16, tag=f"aout{h}", name=f"aout{h}")
                for h in range(HPG)]
        krot = persist.tile([128, T], BF16, tag="krot", name="krot")
        vsd = persist.tile([128, T], BF16, tag="vsd", name="vsd")
        wrm = persist.tile([128, TB], BF16, tag="wrm", name="wrm")

        # ---- weights (packed into single wide tiles, col block = e-chunk) ----
        wpool = ctx.enter_context(tc.tile_pool(name="weights", bufs=1))
        wq_t = wpool.tile([128, NE * 512], BF16, tag="wq", name="wq")
        wk_t = wpool.tile([128, NE * 128], BF16, tag="wk", name="wk")
        wv_t = wpool.tile([128, NE * 128], BF16, tag="wv", name="wv")
        wo_t = wpool.tile([128, NE * 512], BF16, tag="wo", name="wo")

        # ---- psum pools: 2 pools x 2 bufs x 2-bank slots = 8 banks ----
        # P1: projection accumulators + attention score-pairs + outproj eo01
        # P2: rope-swap / v-transpose scratch + attention (pso|psd) + outproj eo23
        P1 = ctx.enter_context(tc.tile_pool(name="P1", bufs=2, space="PSUM"))
        P2 = ctx.enter_context(tc.tile_pool(name="P2", bufs=2, space="PSUM"))

        # ---- sbuf working pools ----
        qrpool = ctx.enter_context(tc.tile_pool(name="qraw", bufs=6))
        ptpool = ctx.enter_context(tc.tile_pool(name="pt", bufs=6))
        ptspool = ctx.enter_context(tc.tile_pool(name="pts", bufs=4))
        srpool = ctx.enter_context(tc.tile_pool(name="sr", bufs=8))
        anpool = ctx.enter_context(tc.tile_pool(name="an", bufs=10))
        ospool = ctx.enter_context(tc.tile_pool(name="osb", bufs=3))

        # ---- input DMAs (few, large, host-prearranged layouts) ----
        # Two HW queues: big streams on sync(SP), constants on scalar(ACT).
        xh3 = ins["xh"].rearrange("p (e t) -> p e t", t=T)

        def load_xblk(t, splits=1):
            dst = xblk[t][:].rearrange("p (e c) -> p e c", c=TB)
            step = NE // splits
            for s in range(splits):
                es = slice(s * step, (s + 1) * step)
                nc.sync.dma_start(dst[:, es, :],
                                  xh3[:, es, t * TB:(t + 1) * TB])

        # sync(SP) queue: x stream + late wq heads + later x blocks
        nc.sync.dma_start(wk_t[:], ins["wk"][:])
        load_xblk(0, splits=8)
        for hq in (2, 3):
            nc.sync.dma_start(wq_t[:, hq * 2048:(hq + 1) * 2048],
                              ins["wq"][:, hq * 2048:(hq + 1) * 2048])
        load_xblk(1, splits=2)
        load_xblk(2)
        load_xblk(3)
        # scalar(ACT) queue: weights/tables stream (few triggers; they
        # head-of-line block the ACT instruction queue once the DMA ring
        # fills, so keep this list short and early)
        nc.scalar.dma_start(swp[:], ins["swp"][:])
        nc.scalar.dma_start(wv_t[:], ins["wv"][:])
        nc.scalar.dma_start(iden[:], ins["iden"][:])
        for hq in (0, 1):
            nc.scalar.dma_start(wq_t[:, hq * 2048:(hq + 1) * 2048],
                                ins["wq"][:, hq * 2048:(hq + 1) * 2048])
        # rope tables: block 0 slice first, rest after the critical window
        nc.scalar.dma_start(cs2[:, 0:TB], ins["cs2"][:, 0:TB])
        nc.scalar.dma_start(snpm[:, 0:TB], ins["snpm"][:, 0:TB])
        nc.scalar.dma_start(tri[:], ins["tri"][:])
        nc.scalar.dma_start(ones[:], ins["onec"][:])
        nc.scalar.dma_start(cs2[:, TB:T], ins["cs2"][:, TB:T])
        nc.scalar.dma_start(snpm[:, TB:T], ins["snpm"][:, TB:T])

        # ---- PE warm-up: dense dummy matmuls during the DMA dead time so
        # the HAM clock-gate reaches 8/8 before real data-paced work starts.
        nc.vector.memset(wrm[:], 0.0)
        pwrm = P2.tile([128, TB], F32, tag="acc", name="pwrm")
        for i in range(42):
            nc.tensor.matmul(pwrm[:], wrm[:, 0:128], wrm[:],
                             start=(i == 0), stop=(i == 41))

        # pending PE work (part-2 of rope / v-transpose), emitted after the
        # NEXT accumulation group's matmuls so the ACT psum->sbuf copy
        # latency hides under queued PE work.
        pend = []

        def flush_pend():
            while pend:
                pend.pop(0)()

        def rope(dst_ap, ps, cols):
            """dst = raw*cos + swap(raw)*sgn_sin, raw in psum ps [128, TB]."""
            qraw = qrpool.tile([128, TB], BF16, tag="qraw", name="qraw")
            # DVE, not ACT: the ACT queue carries the previous attention
            # block's exps and would delay this copy (PE swap-MM stalls)
            nc.vector.tensor_copy(qraw[:], ps[:])

            def part2():
                ps_sw = P2.tile([128, TB], F32, tag="acc", name="psw")
                nc.tensor.matmul(ps_sw[:], swp[:], qraw[:], start=True, stop=True)
                tmp1 = qrpool.tile([128, TB], BF16, tag="qraw", name="ropetmp1")
                nc.vector.tensor_mul(tmp1[:], qraw[:], cs2[:, cols])
                tmp2 = qrpool.tile([128, TB], BF16, tag="qraw", name="ropetmp2")
                nc.vector.tensor_mul(tmp2[:], ps_sw[:], snpm[:, cols])
                nc.vector.tensor_add(dst_ap, tmp1[:], tmp2[:])
            pend.append(part2)

        def project_block(tb):
            cols = slice(tb * TB, (tb + 1) * TB)
            xe = lambda e: xblk[tb][:, e * TB:(e + 1) * TB]

            ps_k = P1.tile([128, TB], F32, tag="ps", name="ps")
            for e in range(NE):
                nc.tensor.matmul(ps_k[:], wk_t[:, e * 128:(e + 1) * 128],
                                 xe(e), start=(e == 0), stop=(e == NE - 1))
            flush_pend()
            rope(krot[:, cols], ps_k, cols)

            ps_v = P1.tile([128, TB], F32, tag="ps", name="ps")
            for e in range(NE):
                nc.tensor.matmul(ps_v[:], wv_t[:, e * 128:(e + 1) * 128],
                                 xe(e), start=(e == 0), stop=(e == NE - 1))
            flush_pend()
            vtmp = qrpool.tile([128, TB], BF16, tag="qraw", name="vtmp")
            nc.vector.tensor_copy(vtmp[:], ps_v[:])

            def vtrans():
                # transpose v tiles of this block: vtmp [d, s] -> vsd [s, d]
                for jj in range(4):
                    j = 4 * tb + jj
                    pst = P2.tile([128, 128], BF16, tag="acc", name="pst")
                    nc.tensor.transpose(pst[:], vtmp[:, jj * 128:(jj + 1) * 128],
                                        iden[:])
                    nc.vector.tensor_copy(vsd[:, j * 128:(j + 1) * 128], pst[:])
            pend.append(vtrans)

            if tb == 0:
                # filler matmuls: bridge the wq DMA wait so HAM stays warm
                pfil = P2.tile([128, TB], F32, tag="acc", name="pfil")
                for i in range(10):
                    nc.tensor.matmul(pfil[:], wrm[:, 0:128], wrm[:],
                                     start=True, stop=True)

            for dq in range(HPG):
                ps = P1.tile([128, TB], F32, tag="ps", name="ps")
                for e in range(NE):
                    nc.tensor.matmul(
                        ps[:],
                        wq_t[:, dq * 2048 + e * 128: dq * 2048 + (e + 1) * 128],
                        xe(e), start=(e == 0), stop=(e == NE - 1))
                flush_pend()
                rope(qrot[dq][:, cols], ps, cols)

        def attention_block(bi):
            jmax = 4 * bi + 3
            npair = (jmax + 1) // 2

            for h in range(HPG):
                acc = P2.tile([128, 2 * TB], F32, tag="acc", name="acc")
                pso = acc[:, 0:TB]       # PV accumulator  (bank A of slot)
                psd = acc[:, TB:2 * TB]  # denominator     (bank B of slot)

                def spair(p):
                    """Two adjacent s-tile score matmuls into one 2-bank tile."""
                    pss = P1.tile([128, 2 * TB], F32, tag="ps", name="pss")
                    info = []
                    for k2 in (0, 1):
                        j = 2 * p + k2
                        o = 128 * (j - 4 * bi) if (j // 4 == bi) else 0
                        nc.tensor.matmul(
                            pss[:, k2 * TB + o:(k2 + 1) * TB],
                            krot[:, j * 128:(j + 1) * 128],
                            qrot[h][:, bi * TB + o:(bi + 1) * TB],
                            start=True, stop=True)
                        info.append((j, o))
                    return pss, info

                cur = spair(0)
                if pend:
                    flush_pend()   # prior rope part-2 rides under h scores
                for p in range(npair):
                    nxt = spair(p + 1) if p + 1 < npair else None
                    pss, info = cur
                    o0 = info[0][1]
                    pt = ptpool.tile([128, 2 * TB], BF16, tag="pt", name="pt")
                    # one exp over both banks (garbage strips are never read)
                    nc.scalar.activation(pt[:, o0:2 * TB], pss[:, o0:2 * TB], EXP)
                    for k2, (j, o) in enumerate(info):
                        kb = k2 * TB
                        if j // 4 == bi:
                            nc.vector.tensor_mul(pt[:, kb + o:kb + o + 128],
                                                 pt[:, kb + o:kb + o + 128], tri[:])
                        nc.tensor.matmul(pso[:, o:TB], vsd[:, j * 128:(j + 1) * 128],
                                         pt[:, kb + o:kb + TB],
                                         start=(j == 0), stop=(j == jmax))
                    # denominator: pre-sum the two halves on DVE (shared
                    # `ones` stationary) -> one matmul per pair
                    oA, oB = info[0][1], info[1][1]
                    first, last = (p == 0), (p == npair - 1)
                    pts = ptspool.tile([128, TB], BF16, tag="pts", name="pts")
                    nc.vector.tensor_add(pts[:, oB:TB], pt[:, oB:TB],
                                         pt[:, TB + oB:2 * TB])
                    if oB > oA:  # strip where only the even half is valid
                        nc.tensor.matmul(psd[:, oA:oB], ones[:], pt[:, oA:oB],
                                         start=first, stop=False)
                        first = False
                    nc.tensor.matmul(psd[:, oB:TB], ones[:], pts[:, oB:TB],
                                     start=first, stop=last)
                    cur = nxt
                cols = slice(bi * TB, (bi + 1) * TB)
                rden = srpool.tile([128, TB], F32, tag="rden", name="rden")
                nc.vector.reciprocal_approx_fast(rden[:], psd)
                nc.vector.tensor_mul(aout[h][:, cols], pso, rden[:])

        # ======== fused projection + attention (per 512-token block) ========
        for tb in range(NTB):
            project_block(tb)
            if tb == 1:
                nc.scalar.dma_start(wo_t[:], ins["wo"][:])
            attention_block(tb)

        # ================= output projection =================
        for tq in range(NST):
            trows = slice(tq * 128, (tq + 1) * 128)
            acc1 = P1.tile([128, 2 * TB], F32, tag="ps", name="po01")
            acc2 = P2.tile([128, 2 * TB], F32, tag="acc", name="po23")
            pos = [acc1[:, 0:TB], acc1[:, TB:2 * TB],
                   acc2[:, 0:TB], acc2[:, TB:2 * TB]]
            osb = ospool.tile([128, 4 * TB], BF16, tag="osb", name="osb")
            if tq < NST - 2:
                for hh in range(HPG):
                    lh = aout[hh][:, trows]
                    for eo in range(4):
                        nc.tensor.matmul(pos[eo],
                                         lh, wo_t[:, (hh * 4 + eo) * 512:(hh * 4 + eo + 1) * 512],
                                         start=(hh == 0), stop=(hh == HPG - 1))
                for eo in range(4):
                    eng = nc.vector.tensor_copy if eo % 2 == 0 else nc.scalar.copy
                    eng(osb[:, eo * TB:(eo + 1) * TB], pos[eo])
                nc.sync.dma_start(out_ap[trows, :], osb[:])
            else:
                # last stripes: eo-outer so each chunk's copy+DMA starts as
                # soon as its own accumulation completes (shorter end chain)
                for eo in range(4):
                    for hh in range(HPG):
                        nc.tensor.matmul(pos[eo],
                                         aout[hh][:, trows],
                                         wo_t[:, (hh * 4 + eo) * 512:(hh * 4 + eo + 1) * 512],
                                         start=(hh == 0), stop=(hh == HPG - 1))
                    eng = nc.vector.tensor_copy if eo % 2 == 0 else nc.scalar.copy
                    eng(osb[:, eo * TB:(eo + 1) * TB], pos[eo])
                    nc.sync.dma_start(out_ap[trows, eo * TB:(eo + 1) * TB],
                                      osb[:, eo * TB:(eo + 1) * TB])


# ---------------- host side ----------------

_PERM = np.concatenate([np.arange(0, HD, 2), np.arange(1, HD, 2)])  # half-split


def _chunked(a, ncols):
    """[E, ncols] -> [128, NE*ncols] with col block e = rows e*128:(e+1)*128."""
    return np.ascontiguousarray(
        a.reshape(NE, 128, ncols).transpose(1, 0, 2).reshape(128, NE * ncols))


def host_prep(inputs):
    """Full inputs -> list of 8 per-core input dicts (core i = (b=i//4, g=i%4))."""
    x = np.asarray(inputs["x"], dtype=np.float32)
    Wq = np.asarray(inputs["Wq"], dtype=np.float32)
    Wk = np.asarray(inputs["Wk"], dtype=np.float32)
    Wv = np.asarray(inputs["Wv"], dtype=np.float32)
    Wo = np.asarray(inputs["Wo"], dtype=np.float32)

    inv = (10000.0 ** (-np.arange(0, HD, 2, dtype=np.float32) / HD)).astype(np.float32)
    tpos = np.arange(T, dtype=np.float32)
    fr = np.outer(tpos, inv)                       # [T, 64]
    cosT = np.cos(fr).T.astype(np.float32)         # [64, T]
    sinT = np.sin(fr).T.astype(np.float32)
    cs2 = np.concatenate([cosT, cosT], axis=0).astype(BF)     # [128, T]
    snpm = np.concatenate([-sinT, sinT], axis=0).astype(BF)   # [128, T]

    tri = (np.arange(128)[None, :] >= np.arange(128)[:, None]).astype(BF)
    swp = np.zeros((128, 128), dtype=np.float32)
    swp[(np.arange(128) + 64) % 128, np.arange(128)] = 1.0
    swp = swp.astype(BF)
    iden = np.eye(128, dtype=np.float32).astype(BF)

    scale = np.float32(1.0 / np.sqrt(HD))
    # x[b].T chunked: xh[p, e*T + t] = x[b][t, e*128+p]
    xh = [_chunked(np.ascontiguousarray(x[b].T), T).astype(BF) for b in range(B)]

    in_maps = []
    for i in range(N_CORES):
        b, g = i // 4, i % 4
        # wq: rows for heads g*4..g*4+3, each permuted, scaled; -> [E, 512]
        rows = []
        for h in range(HPG):
            base = (g * HPG + h) * HD
            rows.append(Wq[base + _PERM, :])
        wq_c = (np.concatenate(rows, axis=0) * scale).T  # [E, 512]
        wk_c = Wk[g * HD + _PERM, :].T                   # [E, 128]
        wv_c = Wv[g * HD:(g + 1) * HD, :].T              # [E, 128]
        wo_c = np.ascontiguousarray(Wo[:, g * 512:(g + 1) * 512].T)  # [512, E]
        # device layout [128, 16*512]: col block (hh*4+eo) = wo_c[hh*128:.., eo*512:..]
        wo_p = np.ascontiguousarray(
            wo_c.reshape(HPG, 128, 4, 512).transpose(1, 0, 2, 3).reshape(128, NE * 512))
        # head-major wq layout: [128, h*2048 + e*128 + c]
        wq_hm = np.concatenate(
            [_chunked(np.ascontiguousarray(wq_c[:, h * 128:(h + 1) * 128]), 128)
             for h in range(HPG)], axis=1)
        in_maps.append({
            "xh": xh[b],
            "wq": wq_hm.astype(BF),
            "wk": _chunked(wk_c, 128).astype(BF),
            "wv": _chunked(wv_c, 128).astype(BF),
            "wo": wo_p.astype(BF),
            "cs2": cs2, "snpm": snpm, "tri": tri, "swp": swp, "iden": iden,
            "onec": np.ones((128, 128), dtype=BF),
        })
    return in_maps


_NC = None


def build_nc():
    global _NC
    if _NC is not None:
        return _NC
    nc = bacc.Bacc("TRN2", target_bir_lowering=False, debug=False,
                   num_devices=N_CORES)
    ins = {
        "xh": nc.dram_tensor("xh", [128, NE * T], BF16, kind="ExternalInput").ap(),
        "wq": nc.dram_tensor("wq", [128, NE * 512], BF16, kind="ExternalInput").ap(),
        "wk": nc.dram_tensor("wk", [128, NE * 128], BF16, kind="ExternalInput").ap(),
        "wv": nc.dram_tensor("wv", [128, NE * 128], BF16, kind="ExternalInput").ap(),
        "wo": nc.dram_tensor("wo", [128, NE * 512], BF16, kind="ExternalInput").ap(),
        "cs2": nc.dram_tensor("cs2", [128, T], BF16, kind="ExternalInput").ap(),
        "snpm": nc.dram_tensor("snpm", [128, T], BF16, kind="ExternalInput").ap(),
        "tri": nc.dram_tensor("tri", [128, 128], BF16, kind="ExternalInput").ap(),
        "swp": nc.dram_tensor("swp", [128, 128], BF16, kind="ExternalInput").ap(),
        "iden": nc.dram_tensor("iden", [128, 128], BF16, kind="ExternalInput").ap(),
        "onec": nc.dram_tensor("onec", [128, 128], BF16, kind="ExternalInput").ap(),
    }
    out = nc.dram_tensor("out", [T, E], BF16, kind="ExternalOutput").ap()
    with tile.TileContext(nc) as tc:
        build_body(tc, out, ins)
    nc.compile()
    _NC = nc
    return nc


def gather(results):
    """results: list of 8 dicts with 'out' [T, E] bf16 partials -> [B, T, E] f32."""
    out = np.zeros((B, T, E), dtype=np.float32)
    for i in range(N_CORES):
        out[i // 4] += np.asarray(results[i]["out"]).astype(np.float32)
    return out


def kernel(**inputs):
    nc = build_nc()
    in_maps = host_prep(inputs)
    res = run_bass_kernel_spmd(nc, in_maps, core_ids=list(range(N_CORES)))
    return gather(res.results)


if __name__ == "__main__":
    rng = np.random.default_rng(0)
    ins = {
        "x": rng.standard_normal((B, T, E), dtype=np.float32),
        "Wq": rng.standard_normal((E, E), dtype=np.float32) * 0.02,
        "Wk": rng.standard_normal((NG * HD, E), dtype=np.float32) * 0.02,
        "Wv": rng.standard_normal((NG * HD, E), dtype=np.float32) * 0.02,
        "Wo": rng.standard_normal((E, E), dtype=np.float32) * 0.02,
    }
    out = kernel(**ins)
    print(out.shape, out.dtype, np.abs(out).mean())


# revision 45
# speedup vs baseline: 1.0084x; 1.0084x over previous
"""GroupedQueryAttention Trainium2 kernel (8 NeuronCores).

Sharding: core i handles (batch b = i//4, KV group g = i%4): its 4 query
heads + 1 KV group, full sequence. Each core computes a partial output
(attn_heads @ Wo rows for its heads); host sums the 4 partials per batch.

v2 layout strategy (per core), all matmul operands bf16 (fp32 PSUM):
  - x resident in SBUF as [128, e-chunk, t] bf16; weights pre-arranged on
    host into exact SBUF layouts so each loads with ONE contiguous DMA.
  - projections W-stationary: qT/kT [d, t] accumulated over 16 e-chunks.
  - RoPE: host permutes W rows to half-split layout; swap-half via a
    permutation matmul on PE; cos/sin tables applied on DVE (bf16 2x).
  - attention interleaved with projections (attention block bi runs right
    after projection block tb=bi): scoresT [s, tq] = kT_tile^T @ qT,
    exp on ACT (scores bounded by construction; no max subtraction),
    denominators via ones-matmul, PV with v[s,d]-stationary accumulation
    -> outT [d, tq]; diagonal blocks narrowed to the causal width.
  - out projection: attn tiles stationary, Wo chunks moving; results
    copied to bf16 SBUF (DVE/ACT alternating) and DMAd per 128-row stripe.
"""

import numpy as np
import ml_dtypes
from contextlib import ExitStack

import concourse.bass as bass
import concourse.bacc as bacc
import concourse.tile as tile
import concourse.mybir as mybir
from concourse.bass_utils import run_bass_kernel_spmd

# problem shape (hardcoded per contract)
B, T, E = 2, 2048, 2048
NH, NG, HD = 16, 4, 128
HPG = NH // NG          # 4 heads per group = per core
NE = E // 128           # 16 contraction chunks
TB = 512                # tq / t block
NTB = T // TB           # 4
NST = T // 128          # 16 t-tiles
F32 = mybir.dt.float32
BF16 = mybir.dt.bfloat16
EXP = mybir.ActivationFunctionType.Exp
COPY = mybir.ActivationFunctionType.Copy

N_CORES = 8
BF = ml_dtypes.bfloat16


def build_body(tc, out_ap, ins):
    """ins: dict name -> dram AP. out_ap: [T, E] bf16 dram AP."""
    nc = tc.nc
    ctx = ExitStack()
    with ctx:
        ctx.enter_context(nc.allow_low_precision(
            reason="bf16 matmul inputs / bf16 intermediate rounding is intended"))
        # ---- constant / persistent SBUF ----
        const = ctx.enter_context(tc.tile_pool(name="const", bufs=1))
        cs2 = const.tile([128, T], BF16, tag="cs2", name="cs2")
        snpm = const.tile([128, T], BF16, tag="snpm", name="snpm")
        tri = const.tile([128, 128], BF16, tag="tri", name="tri")
        swp = const.tile([128, 128], BF16, tag="swp", name="swp")
        iden = const.tile([128, 128], BF16, tag="iden", name="iden")
        ones = const.tile([128, 128], BF16, tag="ones", name="ones")

        persist = ctx.enter_context(tc.tile_pool(name="persist", bufs=1))
        xblk = [persist.tile([128, NE * TB], BF16, tag=f"xb{t}", name=f"xb{t}")
                for t in range(NTB)]
        qrot = [persist.tile([128, T], BF16, tag=f"qrot{h}", name=f"qrot{h}")
                for h in range(HPG)]
        aout = [persist.tile([128, T], BF16, tag=f"aout{h}", name=f"aout{h}")
                for h in range(HPG)]
        krot = persist.tile([128, T], BF16, tag="krot", name="krot")
        vsd = persist.tile([128, T], BF16, tag="vsd", name="vsd")
        wrm = persist.tile([128, TB], BF16, tag="wrm", name="wrm")

        # ---- weights (packed into single wide tiles, col block = e-chunk) ----
        wpool = ctx.enter_context(tc.tile_pool(name="weights", bufs=1))
        wq_t = wpool.tile([128, NE * 512], BF16, tag="wq", name="wq")
        wk_t = wpool.tile([128, NE * 128], BF16, tag="wk", name="wk")
        wv_t = wpool.tile([128, NE * 128], BF16, tag="wv", name="wv")
        wo_t = wpool.tile([128, NE * 512], BF16, tag="wo", name="wo")

        # ---- psum pools: 2 pools x 2 bufs x 2-bank slots = 8 banks ----
        # P1: projection accumulators + attention score-pairs + outproj eo01
        # P2: rope-swap / v-transpose scratch + attention (pso|psd) + outproj eo23
        P1 = ctx.enter_context(tc.tile_pool(name="P1", bufs=2, space="PSUM"))
        P2 = ctx.enter_context(tc.tile_pool(name="P2", bufs=2, space="PSUM"))

        # ---- sbuf working pools ----
        qrpool = ctx.enter_context(tc.tile_pool(name="qraw", bufs=6))
        ptpool = ctx.enter_context(tc.tile_pool(name="pt", bufs=6))
        ptspool = ctx.enter_context(tc.tile_pool(name="pts", bufs=4))
        srpool = ctx.enter_context(tc.tile_pool(name="sr", bufs=8))
        anpool = ctx.enter_context(tc.tile_pool(name="an", bufs=10))
        ospool = ctx.enter_context(tc.tile_pool(name="osb", bufs=3))

        # ---- input DMAs (few, large, host-prearranged layouts) ----
        # Two HW queues: big streams on sync(SP), constants on scalar(ACT).
        xh3 = ins["xh"].rearrange("p (e t) -> p e t", t=T)

        def load_xblk(t, splits=1):
            dst = xblk[t][:].rearrange("p (e c) -> p e c", c=TB)
            step = NE // splits
            for s in range(splits):
                es = slice(s * step, (s + 1) * step)
                nc.sync.dma_start(dst[:, es, :],
                                  xh3[:, es, t * TB:(t + 1) * TB])

        # sync(SP) queue: x stream + late wq heads + later x blocks
        nc.sync.dma_start(wk_t[:], ins["wk"][:])
        load_xblk(0, splits=8)
        for hq in (2, 3):
            nc.sync.dma_start(wq_t[:, hq * 2048:(hq + 1) * 2048],
                              ins["wq"][:, hq * 2048:(hq + 1) * 2048])
        load_xblk(1, splits=2)
        load_xblk(2)
        load_xblk(3)
        # scalar(ACT) queue: weights/tables stream (few triggers; they
        # head-of-line block the ACT instruction queue once the DMA ring
        # fills, so keep this list short and early)
        nc.scalar.dma_start(swp[:], ins["swp"][:])
        nc.scalar.dma_start(wv_t[:], ins["wv"][:])
        nc.scalar.dma_start(iden[:], ins["iden"][:])
        for hq in (0, 1):
            nc.scalar.dma_start(wq_t[:, hq * 2048:(hq + 1) * 2048],
                                ins["wq"][:, hq * 2048:(hq + 1) * 2048])
        # rope tables: block 0 slice first, rest after the critical window
        nc.scalar.dma_start(cs2[:, 0:TB], ins["cs2"][:, 0:TB])
        nc.scalar.dma_start(snpm[:, 0:TB], ins["snpm"][:, 0:TB])
        nc.scalar.dma_start(tri[:], ins["tri"][:])
        nc.scalar.dma_start(ones[:], ins["onec"][:])
        nc.scalar.dma_start(cs2[:, TB:T], ins["cs2"][:, TB:T])
        nc.scalar.dma_start(snpm[:, TB:T], ins["snpm"][:, TB:T])

        # ---- PE warm-up: dense dummy matmuls during the DMA dead time so
        # the HAM clock-gate reaches 8/8 before real data-paced work starts.
        nc.vector.memset(wrm[:], 0.0)
        pwrm = P2.tile([128, TB], F32, tag="acc", name="pwrm")
        for i in range(42):
            nc.tensor.matmul(pwrm[:], wrm[:, 0:128], wrm[:],
                             start=(i == 0), stop=(i == 41))

        # pending PE work (part-2 of rope / v-transpose), emitted after the
        # NEXT accumulation group's matmuls so the ACT psum->sbuf copy
        # latency hides under queued PE work.
        pend = []

        def flush_pend():
            while pend:
                pend.pop(0)()

        def rope(dst_ap, ps, cols):
            """dst = raw*cos + swap(raw)*sgn_sin, raw in psum ps [128, TB]."""
            qraw = qrpool.tile([128, TB], BF16, tag="qraw", name="qraw")
            nc.scalar.copy(qraw[:], ps[:])

            def part2():
                ps_sw = P2.tile([128, TB], F32, tag="acc", name="psw")
                nc.tensor.matmul(ps_sw[:], swp[:], qraw[:], start=True, stop=True)
                tmp1 = qrpool.tile([128, TB], BF16, tag="qraw", name="ropetmp1")
                nc.vector.tensor_mul(tmp1[:], qraw[:], cs2[:, cols])
                tmp2 = qrpool.tile([128, TB], BF16, tag="qraw", name="ropetmp2")
                nc.vector.tensor_mul(tmp2[:], ps_sw[:], snpm[:, cols])
                nc.vector.tensor_add(dst_ap, tmp1[:], tmp2[:])
            pend.append(part2)

        def project_block(tb):
            cols = slice(tb * TB, (tb + 1) * TB)
            xe = lambda e: xblk[tb][:, e * TB:(e + 1) * TB]

            ps_k = P1.tile([128, TB], F32, tag="ps", name="ps")
            for e in range(NE):
                nc.tensor.matmul(ps_k[:], wk_t[:, e * 128:(e + 1) * 128],
                                 xe(e), start=(e == 0), stop=(e == NE - 1))
            flush_pend()
            rope(krot[:, cols], ps_k, cols)

            ps_v = P1.tile([128, TB], F32, tag="ps", name="ps")
            for e in range(NE):
                nc.tensor.matmul(ps_v[:], wv_t[:, e * 128:(e + 1) * 128],
                                 xe(e), start=(e == 0), stop=(e == NE - 1))
            flush_pend()
            vtmp = qrpool.tile([128, TB], BF16, tag="qraw", name="vtmp")
            nc.scalar.copy(vtmp[:], ps_v[:])

            def vtrans():
                # transpose v tiles of this block: vtmp [d, s] -> vsd [s, d]
                for jj in range(4):
                    j = 4 * tb + jj
                    pst = P2.tile([128, 128], BF16, tag="acc", name="pst")
                    nc.tensor.transpose(pst[:], vtmp[:, jj * 128:(jj + 1) * 128],
                                        iden[:])
                    nc.vector.tensor_copy(vsd[:, j * 128:(j + 1) * 128], pst[:])
            pend.append(vtrans)

            if tb == 0:
                # filler matmuls: bridge the wq DMA wait so HAM stays warm
                pfil = P2.tile([128, TB], F32, tag="acc", name="pfil")
                for i in range(10):
                    nc.tensor.matmul(pfil[:], wrm[:, 0:128], wrm[:],
                                     start=True, stop=True)

            for dq in range(HPG):
                ps = P1.tile([128, TB], F32, tag="ps", name="ps")
                for e in range(NE):
                    nc.tensor.matmul(
                        ps[:],
                        wq_t[:, dq * 2048 + e * 128: dq * 2048 + (e + 1) * 128],
                        xe(e), start=(e == 0), stop=(e == NE - 1))
                flush_pend()
                rope(qrot[dq][:, cols], ps, cols)

        def attention_block(bi):
            jmax = 4 * bi + 3
            npair = (jmax + 1) // 2

            for h in range(HPG):
                acc = P2.tile([128, 2 * TB], F32, tag="acc", name="acc")
                pso = acc[:, 0:TB]       # PV accumulator  (bank A of slot)
                psd = acc[:, TB:2 * TB]  # denominator     (bank B of slot)

                def spair(p):
                    """Two adjacent s-tile score matmuls into one 2-bank tile."""
                    pss = P1.tile([128, 2 * TB], F32, tag="ps", name="pss")
                    info = []
                    for k2 in (0, 1):
                        j = 2 * p + k2
                        o = 128 * (j - 4 * bi) if (j // 4 == bi) else 0
                        nc.tensor.matmul(
                            pss[:, k2 * TB + o:(k2 + 1) * TB],
                            krot[:, j * 128:(j + 1) * 128],
                            qrot[h][:, bi * TB + o:(bi + 1) * TB],
                            start=True, stop=True)
                        info.append((j, o))
                    return pss, info

                cur = spair(0)
                if pend:
                    flush_pend()   # prior rope part-2 rides under h scores
                for p in range(npair):
                    nxt = spair(p + 1) if p + 1 < npair else None
                    pss, info = cur
                    o0 = info[0][1]
                    pt = ptpool.tile([128, 2 * TB], BF16, tag="pt", name="pt")
                    # one exp over both banks (garbage strips are never read)
                    nc.scalar.activation(pt[:, o0:2 * TB], pss[:, o0:2 * TB], EXP)
                    for k2, (j, o) in enumerate(info):
                        kb = k2 * TB
                        if j // 4 == bi:
                            nc.vector.tensor_mul(pt[:, kb + o:kb + o + 128],
                                                 pt[:, kb + o:kb + o + 128], tri[:])
                        nc.tensor.matmul(pso[:, o:TB], vsd[:, j * 128:(j + 1) * 128],
                                         pt[:, kb + o:kb + TB],
                                         start=(j == 0), stop=(j == jmax))
                    # denominator: pre-sum the two halves on DVE (shared
                    # `ones` stationary) -> one matmul per pair
                    oA, oB = info[0][1], info[1][1]
                    first, last = (p == 0), (p == npair - 1)
                    pts = ptspool.tile([128, TB], BF16, tag="pts", name="pts")
                    nc.vector.tensor_add(pts[:, oB:TB], pt[:, oB:TB],
                                         pt[:, TB + oB:2 * TB])
                    if oB > oA:  # strip where only the even half is valid
                        nc.tensor.matmul(psd[:, oA:oB], ones[:], pt[:, oA:oB],
                                         start=first, stop=False)
                        first = False
                    nc.tensor.matmul(psd[:, oB:TB], ones[:], pts[:, oB:TB],
                                     start=first, stop=last)
                    cur = nxt
                cols = slice(bi * TB, (bi + 1) * TB)
                rden = srpool.tile([128, TB], F32, tag="rden", name="rden")
                nc.vector.reciprocal_approx_fast(rden[:], psd)
                nc.vector.tensor_mul(aout[h][:, cols], pso, rden[:])

        # ======== fused projection + attention (per 512-token block) ========
        for tb in range(NTB):
            project_block(tb)
            if tb == 1:
                nc.scalar.dma_start(wo_t[:], ins["wo"][:])
            attention_block(tb)

        # ================= output projection =================
        for tq in range(NST):
            trows = slice(tq * 128, (tq + 1) * 128)
            acc1 = P1.tile([128, 2 * TB], F32, tag="ps", name="po01")
            acc2 = P2.tile([128, 2 * TB], F32, tag="acc", name="po23")
            pos = [acc1[:, 0:TB], acc1[:, TB:2 * TB],
                   acc2[:, 0:TB], acc2[:, TB:2 * TB]]
            osb = ospool.tile([128, 4 * TB], BF16, tag="osb", name="osb")
            if tq < NST - 2:
                for hh in range(HPG):
                    lh = aout[hh][:, trows]
                    for eo in range(4):
                        nc.tensor.matmul(pos[eo],
                                         lh, wo_t[:, (hh * 4 + eo) * 512:(hh * 4 + eo + 1) * 512],
                                         start=(hh == 0), stop=(hh == HPG - 1))
                for eo in range(4):
                    eng = nc.vector.tensor_copy if eo % 2 == 0 else nc.scalar.copy
                    eng(osb[:, eo * TB:(eo + 1) * TB], pos[eo])
                nc.sync.dma_start(out_ap[trows, :], osb[:])
            else:
                # last stripes: eo-outer so each chunk's copy+DMA starts as
                # soon as its own accumulation completes (shorter end chain)
                for eo in range(4):
                    for hh in range(HPG):
                        nc.tensor.matmul(pos[eo],
                                         aout[hh][:, trows],
                                         wo_t[:, (hh * 4 + eo) * 512:(hh * 4 + eo + 1) * 512],
                                         start=(hh == 0), stop=(hh == HPG - 1))
                    eng = nc.vector.tensor_copy if eo % 2 == 0 else nc.scalar.copy
                    eng(osb[:, eo * TB:(eo + 1) * TB], pos[eo])
                    nc.sync.dma_start(out_ap[trows, eo * TB:(eo + 1) * TB],
                                      osb[:, eo * TB:(eo + 1) * TB])


# ---------------- host side ----------------

_PERM = np.concatenate([np.arange(0, HD, 2), np.arange(1, HD, 2)])  # half-split


def _chunked(a, ncols):
    """[E, ncols] -> [128, NE*ncols] with col block e = rows e*128:(e+1)*128."""
    return np.ascontiguousarray(
        a.reshape(NE, 128, ncols).transpose(1, 0, 2).reshape(128, NE * ncols))


def host_prep(inputs):
    """Full inputs -> list of 8 per-core input dicts (core i = (b=i//4, g=i%4))."""
    x = np.asarray(inputs["x"], dtype=np.float32)
    Wq = np.asarray(inputs["Wq"], dtype=np.float32)
    Wk = np.asarray(inputs["Wk"], dtype=np.float32)
    Wv = np.asarray(inputs["Wv"], dtype=np.float32)
    Wo = np.asarray(inputs["Wo"], dtype=np.float32)

    inv = (10000.0 ** (-np.arange(0, HD, 2, dtype=np.float32) / HD)).astype(np.float32)
    tpos = np.arange(T, dtype=np.float32)
    fr = np.outer(tpos, inv)                       # [T, 64]
    cosT = np.cos(fr).T.astype(np.float32)         # [64, T]
    sinT = np.sin(fr).T.astype(np.float32)
    cs2 = np.concatenate([cosT, cosT], axis=0).astype(BF)     # [128, T]
    snpm = np.concatenate([-sinT, sinT], axis=0).astype(BF)   # [128, T]

    tri = (np.arange(128)[None, :] >= np.arange(128)[:, None]).astype(BF)
    swp = np.zeros((128, 128), dtype=np.float32)
    swp[(np.arange(128) + 64) % 128, np.arange(128)] = 1.0
    swp = swp.astype(BF)
    iden = np.eye(128, dtype=np.float32).astype(BF)

    scale = np.float32(1.0 / np.sqrt(HD))
    # x[b].T chunked: xh[p, e*T + t] = x[b][t, e*128+p]
    xh = [_chunked(np.ascontiguousarray(x[b].T), T).astype(BF) for b in range(B)]

    in_maps = []
    for i in range(N_CORES):
        b, g = i // 4, i % 4
        # wq: rows for heads g*4..g*4+3, each permuted, scaled; -> [E, 512]
        rows = []
        for h in range(HPG):
            base = (g * HPG + h) * HD
            rows.append(Wq[base + _PERM, :])
        wq_c = (np.concatenate(rows, axis=0) * scale).T  # [E, 512]
        wk_c = Wk[g * HD + _PERM, :].T                   # [E, 128]
        wv_c = Wv[g * HD:(g + 1) * HD, :].T              # [E, 128]
        wo_c = np.ascontiguousarray(Wo[:, g * 512:(g + 1) * 512].T)  # [512, E]
        # device layout [128, 16*512]: col block (hh*4+eo) = wo_c[hh*128:.., eo*512:..]
        wo_p = np.ascontiguousarray(
            wo_c.reshape(HPG, 128, 4, 512).transpose(1, 0, 2, 3).reshape(128, NE * 512))
        # head-major wq layout: [128, h*2048 + e*128 + c]
        wq_hm = np.concatenate(
            [_chunked(np.ascontiguousarray(wq_c[:, h * 128:(h + 1) * 128]), 128)
             for h in range(HPG)], axis=1)
        in_maps.append({
            "xh": xh[b],
            "wq": wq_hm.astype(BF),
            "wk": _chunked(wk_c, 128).astype(BF),
            "wv": _chunked(wv_c, 128).astype(BF),
            "wo": wo_p.astype(BF),
            "cs2": cs2, "snpm": snpm, "tri": tri, "swp": swp, "iden": iden,
            "onec": np.ones((128, 128), dtype=BF),
        })
    return in_maps


_NC = None


def build_nc():
    global _NC
    if _NC is not None:
        return _NC
    nc = bacc.Bacc("TRN2", target_bir_lowering=False, debug=False,
                   num_devices=N_CORES)
    ins = {
        "xh": nc.dram_tensor("xh", [128, NE * T], BF16, kind="ExternalInput").ap(),
        "wq": nc.dram_tensor("wq", [128, NE * 512], BF16, kind="ExternalInput").ap(),
        "wk": nc.dram_tensor("wk", [128, NE * 128], BF16, kind="ExternalInput").ap(),
        "wv": nc.dram_tensor("wv", [128, NE * 128], BF16, kind="ExternalInput").ap(),
        "wo": nc.dram_tensor("wo", [128, NE * 512], BF16, kind="ExternalInput").ap(),
        "cs2": nc.dram_tensor("cs2", [128, T], BF16, kind="ExternalInput").ap(),
        "snpm": nc.dram_tensor("snpm", [128, T], BF16, kind="ExternalInput").ap(),
        "tri": nc.dram_tensor("tri", [128, 128], BF16, kind="ExternalInput").ap(),
        "swp": nc.dram_tensor("swp", [128, 128], BF16, kind="ExternalInput").ap(),
        "iden": nc.dram_tensor("iden", [128, 128], BF16, kind="ExternalInput").ap(),
        "onec": nc.dram_tensor("onec", [128, 128], BF16, kind="ExternalInput").ap(),
    }
    out = nc.dram_tensor("out", [T, E], BF16, kind="ExternalOutput").ap()
    with tile.TileContext(nc) as tc:
        build_body(tc, out, ins)
    nc.compile()
    _NC = nc
    return nc


def gather(results):
    """results: list of 8 dicts with 'out' [T, E] bf16 partials -> [B, T, E] f32."""
    out = np.zeros((B, T, E), dtype=np.float32)
    for i in range(N_CORES):
        out[i // 4] += np.asarray(results[i]["out"]).astype(np.float32)
    return out


def kernel(**inputs):
    nc = build_nc()
    in_maps = host_prep(inputs)
    res = run_bass_kernel_spmd(nc, in_maps, core_ids=list(range(N_CORES)))
    return gather(res.results)


if __name__ == "__main__":
    rng = np.random.default_rng(0)
    ins = {
        "x": rng.standard_normal((B, T, E), dtype=np.float32),
        "Wq": rng.standard_normal((E, E), dtype=np.float32) * 0.02,
        "Wk": rng.standard_normal((NG * HD, E), dtype=np.float32) * 0.02,
        "Wv": rng.standard_normal((NG * HD, E), dtype=np.float32) * 0.02,
        "Wo": rng.standard_normal((E, E), dtype=np.float32) * 0.02,
    }
    out = kernel(**ins)
    print(out.shape, out.dtype, np.abs(out).mean())
